# revision 56
# baseline (speedup 1.0000x reference)
"""Distributed causal multi-head attention layer for one TRN2 chip (8 NeuronCores).

Problem: S=2048, B=4, D=512, H=8 heads (DH=64), causal mask, fp32.

Sharding: core c handles batch b = c//2 and heads [4*(c%2), 4*(c%2)+4).
Each core computes its 4 heads' attention output for its batch; the host
concatenates per-core outputs (no cross-core collectives needed).

Per-core kernel (Tile framework):
  - QKV projections on TensorE (float32r = full-rate fp32 path):
      qT/kT produced in [dh, seq] layout (2 heads stacked per 128 partitions),
      v produced in [seq, dh] layout with a ones-column appended (col 64).
  - Attention per head, q swept in 2 half-rows of 1024:
      scoresT[k,q] tile = kT_head(128 keys) x qT_head  (PE)
      causal tri-mask add on the diagonal 128x128 block (DVE)
      w = exp(scores/8)  (ScalarE, PSUM->SBUF)
      out_aug[65, q] += v_aug.T @ w  (PE; row 64 accumulates softmax denom)
  - Epilogue per sweep: denom -> reciprocal -> partition-broadcast (GPSIMD),
      multiply + bias add (DVE), DMA out in [dh, seq] layout.
Host transposes/concats per-head blocks into the full [S, B, D] output.
"""

import numpy as np

import concourse.bass as bass
import concourse.tile as tile
from concourse import bacc, mybir
from concourse.bass_utils import run_bass_kernel_spmd

S, B, D, H = 2048, 4, 512, 8
DH = D // H            # 64
HPC = 4                # heads per core
NCORE = 8
SW = 1024              # q sweep width
NSW = S // SW          # 2
KT = 128               # key tile (partition dim)
NEG = np.float32(-1e9)

F32 = mybir.dt.float32
F32R = mybir.dt.float32r
BF16 = mybir.dt.bfloat16


def build_nc(causal: bool, reps: int = 0) -> bacc.Bacc:
    """reps>0 wraps the whole body in a hardware loop (for on-device timing)."""
    nc = bacc.Bacc("TRN2", target_bir_lowering=False, debug=False, num_devices=NCORE)

    xT = nc.declare_dram_parameter("xT", [D, S], F32R, isOutput=False)
    kxT = nc.declare_dram_parameter("kxT", [D, S], F32R, isOutput=False)
    vxT = nc.declare_dram_parameter("vxT", [D, S], F32R, isOutput=False)
    wv = nc.declare_dram_parameter("wv", [D, HPC * DH], F32R, isOutput=False)
    wqk = nc.declare_dram_parameter("wqk", [2, D, HPC * DH], F32R, isOutput=False)
    # constants blob: [128, 136] = tri(0:128) | bqT(128:130) | bkT(130:132) | bvT(132:136)
    cst = nc.declare_dram_parameter("cst", [128, 136], F32, isOutput=False)
    out = nc.declare_dram_parameter("out", [HPC, DH, S], F32, isOutput=True)

    NDC = D // 128  # 4 d-chunks

    from contextlib import ExitStack
    with tile.TileContext(nc) as tc, ExitStack() as _st:
        persist = _st.enter_context(tc.tile_pool(name="persist", bufs=1))
        wpool = _st.enter_context(tc.tile_pool(name="wtile", bufs=8))
        rpool = _st.enter_context(tc.tile_pool(name="res", bufs=3))
        eppool = _st.enter_context(tc.tile_pool(name="eptmp", bufs=2))
        ps_sc = _st.enter_context(tc.tile_pool(name="ps_sc", bufs=3, space="PSUM"))
        ps_out = _st.enter_context(tc.tile_pool(name="ps_out", bufs=2, space="PSUM"))
        if reps:
            _st.enter_context(tc.For_i(0, reps, 1))
        if True:
            # ---- constants + weights: consolidated single DMAs ----
            cst_sb = persist.tile([128, 136], F32, tag="cst")
            nc.scalar.dma_start(out=cst_sb[:], in_=cst[:])
            tri_sb = cst_sb[:, 0:KT]
            bq_sb = cst_sb[:, 128:130]
            bk_sb = cst_sb[:, 130:132]
            bv_sb = cst_sb[0:DH, 132:136]

            wv_sb = persist.tile([128, NDC, HPC * DH], F32R, tag="wv")
            nc.scalar.dma_start(
                out=wv_sb[:], in_=wv.rearrange("(dc p) j -> p dc j", p=128))
            # wqk gates every projection matmul: first on the sync queue
            wqk_sb = persist.tile([128, 2, NDC, HPC * DH], F32R, tag="wqk")
            nc.sync.dma_start(
                out=wqk_sb[:], in_=wqk.rearrange("t (dc p) j -> p t dc j", p=128))
            wq_sb = wqk_sb[:, 0]
            wk_sb = wqk_sb[:, 1]

            x_sb = persist.tile([128, NDC, S], F32R, tag="x")
            kx_sb = persist.tile([128, NDC, S], F32R, tag="kx")
            qT_sb = persist.tile([128, 2, S], BF16, tag="qT")
            kT_sb = object()  # sentinel for the eviction branch
            # per-head K-padded key tiles: complement rows are zero so
            # scores matmuls run at K=128 (fast weight-load path)
            kTz_sb = persist.tile([128, HPC, S], BF16, tag="kTz")
            v_sb = persist.tile([128, S // 128, HPC, DH + 1], BF16, tag="v")


            vxpool = _st.enter_context(tc.tile_pool(name="vxp", bufs=2))
            _vq = {}

            def vx_dma(qi):
                # DMA one 512-seq quarter of vx (issued early; projected later)
                vq = vxpool.tile([128, NDC, 512], F32R, tag="vxs")
                _vq[qi] = vq
                vxr = vxT.rearrange("(dc p) s -> p dc s", p=128)
                nc.sync.dma_start(out=vq[:], in_=vxr[:, :, qi * 512:(qi + 1) * 512])

            def v_proj(qi):
                vq = _vq.pop(qi)
                for st4 in range(4):
                    st = qi * 4 + st4
                    ps = ps_sc.tile([128, SW], F32, tag="sc")
                    for dc in range(NDC):
                        nc.tensor.matmul(
                            ps[:, 0:HPC * DH],
                            vq[:, dc, st4 * 128:(st4 + 1) * 128],
                            wv_sb[:, dc, :],
                            start=(dc == 0),
                            stop=(dc == NDC - 1),
                        )
                    nc.vector.tensor_copy(
                        out=v_sb[:, st, :, 0:DH],
                        in_=ps[:, 0:HPC * DH].rearrange("p (u d) -> p u d", u=HPC),
                    )

            def proj_dma(s0):
                xr = xT.rearrange("(dc p) s -> p dc s", p=128)
                kxr = kxT.rearrange("(dc p) s -> p dc s", p=128)
                for q in range(s0, s0 + SW, 512):
                    nc.sync.dma_start(out=x_sb[:, :, q:q + 512], in_=xr[:, :, q:q + 512])
                    nc.sync.dma_start(out=kx_sb[:, :, q:q + 512], in_=kxr[:, :, q:q + 512])

            def proj_half(s0):
                # g outer: head-group 0's q AND k finish first (they gate
                # the first two attention units)
                for g in range(2):
                    for (w_sb, b_sb, src, dst) in (
                        (wq_sb, bq_sb, x_sb, qT_sb), (wk_sb, bk_sb, kx_sb, kT_sb)
                    ):
                        ps = ps_sc.tile([128, SW], F32, tag="sc")
                        for nchunk in range(0, SW, 512):
                            for dc in range(NDC):
                                nc.tensor.matmul(
                                    ps[:, nchunk:nchunk + 512],
                                    w_sb[:, dc, g * 128:(g + 1) * 128],
                                    src[:, dc, s0 + nchunk:s0 + nchunk + 512],
                                    start=(dc == 0),
                                    stop=(dc == NDC - 1),
                                )
                            # per-512 bias add releases qT/kT chunks early
                            ch = slice(s0 + nchunk, s0 + nchunk + 512)
                            if dst is kT_sb:
                                # k rows land in the SAME row range as the
                                # head's q rows; complement rows are zero
                                for ho in range(2):
                                    rs = slice(ho * DH, (ho + 1) * DH)
                                    nc.vector.tensor_scalar_add(
                                        out=kTz_sb[rs, 2 * g + ho, ch],
                                        in0=ps[rs, nchunk:nchunk + 512],
                                        scalar1=b_sb[rs, g:g + 1],
                                    )
                            else:
                                nc.vector.tensor_scalar_add(
                                    out=dst[:, g, ch],
                                    in0=ps[:, nchunk:nchunk + 512],
                                    scalar1=b_sb[:, g:g + 1],
                                )

            def attn_sweep(u, sw):
                g, ho = u // 2, u % 2
                qh = qT_sb[:, g, :]       # [128, S]; rows 64+ hit zero weights
                kh = kTz_sb[:, u, :]      # [128, S] zero-padded per head
                q0 = sw * SW
                qw = SW
                nkt = (q0 + qw) // KT if causal else S // KT
                ncc = qw // 512
                # one accumulator (1 PSUM bank) per 512-q-chunk: earlier chunks
                # finish at earlier k-tiles, freeing slots sooner
                o_ps = []
                for _cc in range(ncc):
                    o_chunk = ps_out.tile([DH + 1, 512], F32, tag="out")
                    o_ps.append(o_chunk)
                # last k-tile contributing to each 512-chunk of the sweep
                last_kt = [
                    min(nkt - 1, (q0 + ch + 512 - 1) // KT)
                    for ch in range(0, qw, 512)
                ] if causal else [nkt - 1] * (qw // 512)
                def emit_av(kt, w, a0):
                    for c0 in range(a0, qw, 512):
                        c1 = min(c0 + 512, qw)
                        nc.tensor.matmul(
                            o_ps[c0 // 512][:, 0:c1 - c0],
                            v_sb[:, kt, u, :],
                            w[:, c0:c1],
                            start=(kt == 0),
                            stop=(kt == last_kt[c0 // 512]),
                        )

                pend = None   # software-pipeline AV one k-tile behind scores
                for kt in range(nkt):
                    o = max(0, kt * KT - q0) if causal else 0
                    a0 = (o // 512) * 512              # 512-aligned start for AV
                    sc = ps_sc.tile([128, SW], F32, tag="sc")
                    c0 = o
                    while c0 < qw:
                        c1 = min(((c0 // 512) + 1) * 512, qw)
                        nc.tensor.matmul(
                            sc[:, c0:c1],
                            kh[:, kt * KT:(kt + 1) * KT],
                            qh[:, q0 + c0:q0 + c1],
                            start=True, stop=True,
                        )
                        c0 = c1
                    if causal and kt * KT >= q0:
                        # diagonal block: cols [o, o+128)
                        nc.vector.tensor_add(
                            out=sc[:, o:o + KT], in0=sc[:, o:o + KT], in1=tri_sb[:]
                        )
                    w = wpool.tile([128, SW], BF16, tag="w")
                    if o > a0:
                        nc.gpsimd.memset(w[:, a0:o], 0.0)
                    nc.scalar.activation(
                        out=w[:, o:qw], in_=sc[:, o:qw],
                        func=mybir.ActivationFunctionType.Exp, scale=0.125,
                    )
                    if pend is not None:
                        emit_av(*pend)
                    pend = (kt, w, a0)
                if pend is not None:
                    emit_av(*pend)
                # epilogue per chunk: divide by denoms (row 64) + bias, DMA out
                for cc in range(ncc):
                    op = o_ps[cc]
                    r65 = eppool.tile([DH + 1, 512], F32, tag="r65")
                    nc.vector.reciprocal(out=r65[DH:DH + 1, :], in_=op[DH:DH + 1, :])
                    r0 = eppool.tile([1, 512], F32, tag="r0")
                    nc.sync.dma_start(out=r0[:], in_=r65[DH:DH + 1, :])
                    db = eppool.tile([DH, 512], F32, tag="db")
                    nc.gpsimd.partition_broadcast(db[:], r0[:])
                    res = rpool.tile([DH, 512], F32, tag="res")
                    nc.vector.tensor_mul(out=res[:], in0=op[0:DH, :], in1=db[:])
                    nc.vector.tensor_scalar_add(
                        out=res[:], in0=res[:], scalar1=bv_sb[:, u:u + 1])
                    nc.sync.dma_start(
                        out=out[u, :, q0 + cc * 512:q0 + (cc + 1) * 512], in_=res[:])

            # sweep 0 only needs the first half of qT/kT: interleave so
            # attention starts while half-1 inputs are still in flight.
            # v ones column (bv added at the very end)
            nc.vector.memset(v_sb[:, :, :, DH], 1.0)
            for _u in range(HPC):
                _zr = slice(DH, 128) if _u % 2 == 0 else slice(0, DH)
                nc.gpsimd.memset(kTz_sb[_zr, _u, :], 0.0)
            proj_dma(0)
            proj_half(0)
            vx_dma(0)
            v_proj(0)
            vx_dma(1)
            v_proj(1)            # v for k-tiles 0..7 (all sweep-0 needs)
            proj_dma(SW)         # enqueue ALL remaining input loads before
            vx_dma(2)            # any compute-gated epilogue DMA
            vx_dma(3)
            attn_sweep(0, 0)
            attn_sweep(1, 0)
            proj_half(SW)
            attn_sweep(2, 0)
            attn_sweep(3, 0)
            v_proj(2)
            v_proj(3)            # v for k-tiles 8..15 (sweep 1)
            for u in range(HPC):
                attn_sweep(u, 1)

    nc.finalize()
    return nc


_NC_CACHE = {}


def _get_nc(causal: bool):
    if causal not in _NC_CACHE:
        _NC_CACHE[causal] = build_nc(causal)
    return _NC_CACHE[causal]


def make_in_maps(input_tensor, keys_vector, values_vector, Wq, bq, Wk, bk, Wv, bv):
    # scores tiles are [k, q] (transposed): keep k <= q  ->  upper triangle
    tri_np = np.where(
        np.triu(np.ones((KT, KT), dtype=bool)), np.float32(0), NEG
    ).astype(np.float32)
    in_maps = []
    for c in range(NCORE):
        b, hg = c // 2, c % 2
        hs = slice(hg * HPC * DH, (hg + 1) * HPC * DH)
        cst = np.zeros((128, 136), np.float32)
        cst[:, 0:128] = tri_np
        cst[:, 128:130] = np.asarray(bq)[hs].reshape(2, 128).T
        cst[:, 130:132] = np.asarray(bk)[hs].reshape(2, 128).T
        cst[0:DH, 132:136] = np.asarray(bv)[hs].reshape(HPC, DH).T
        m = {
            "xT": np.ascontiguousarray(np.asarray(input_tensor)[:, b, :].T),
            "kxT": np.ascontiguousarray(np.asarray(keys_vector)[:, b, :].T),
            "vxT": np.ascontiguousarray(np.asarray(values_vector)[:, b, :].T),
            "wv": np.ascontiguousarray(np.asarray(Wv)[:, hs]),
            "wqk": np.ascontiguousarray(
                np.stack([np.asarray(Wq)[:, hs], np.asarray(Wk)[:, hs]])),
            "cst": cst,
        }
        in_maps.append(m)
    return in_maps


def assemble_output(results):
    full = np.empty((S, B, D), dtype=np.float32)
    for c in range(NCORE):
        b, hg = c // 2, c % 2
        o = results[c]["out"]  # [HPC, DH, S]
        for u in range(HPC):
            h = hg * HPC + u
            full[:, b, h * DH:(h + 1) * DH] = o[u].T
    return full


def kernel(input_tensor, keys_vector, values_vector, Wq, bq, Wk, bk, Wv, bv, mask):
    causal = bool(np.asarray(mask).item()) if np.asarray(mask).size == 1 else True
    nc = _get_nc(causal)
    in_maps = make_in_maps(
        input_tensor, keys_vector, values_vector, Wq, bq, Wk, bk, Wv, bv
    )
    res = run_bass_kernel_spmd(nc, in_maps, core_ids=list(range(NCORE)))
    return assemble_output(res.results)


# revision 58
# speedup vs baseline: 1.1063x; 1.1063x over previous
"""Distributed causal multi-head attention layer for one TRN2 chip (8 NeuronCores).

Problem: S=2048, B=4, D=512, H=8 heads (DH=64), causal mask, fp32 I/O.

Sharding: core c handles batch b = c//2 and heads [4*(c%2), 4*(c%2)+4).
Each core computes its 4 heads' attention for its batch; the host
concatenates per-core outputs (no cross-core collectives needed).

Per-core kernel (Tile framework), flash-attention style without max-subtraction
(scores ~ N(0,1), fp32 exp cannot overflow):
  - QKV projections on TensorE in float32r (full-rate fp32): qT in [dh, seq]
    layout (2 heads per 128 partitions), per-head zero-padded kTz (bf16), and
    v in [seq, dh] bf16 with a ones-column at col 64.
  - K is zero-padded to 128 per head (complement rows zero) so scores matmuls
    run at K=128: the K=64 fp32r path measured 507 ns/matmul on HW vs 365 for
    K=128 bf16 (LDWEIGHTS fast-path).
  - Attention per head, q swept in 2 half-rows of 1024, k-tiles of 128:
      scoresT[k,q] = kTz_head x qT (PE, bf16, fp32 PSUM)
      causal tri-mask add on the diagonal 128x128 block (DVE)
      w = exp(scores/8) (ScalarE, PSUM -> bf16 SBUF)
      out_aug[65, 512-chunk] += v_aug.T @ w (PE; row 64 = softmax denominator)
  - Epilogue per 512-chunk: reciprocal (DVE) -> DMA shift to partition 0 ->
    partition_broadcast (GPSIMD reads physical partition 0 only!) -> multiply
    + bias add (DVE) -> DMA out in [dh, seq] layout.
  - DMA choreography: weights first, then x/kx half-0 quarters, vx, half-1 --
    all input DMAs enqueue on the sync queue before any compute-gated epilogue
    DMA (FIFO inversion otherwise delays half-1 inputs by ~10 us).
Host transposes/concats per-head blocks into the full [S, B, D] output.
reps>0 wraps the body in a hardware For_i loop for on-device timing.
"""

import numpy as np

import concourse.bass as bass
import concourse.tile as tile
from concourse import bacc, mybir
from concourse.bass_utils import run_bass_kernel_spmd

S, B, D, H = 2048, 4, 512, 8
DH = D // H            # 64
HPC = 4                # heads per core
NCORE = 8
SW = 1024              # q sweep width
NSW = S // SW          # 2
KT = 128               # key tile (partition dim)
NEG = np.float32(-1e9)

F32 = mybir.dt.float32
F32R = mybir.dt.float32r
BF16 = mybir.dt.bfloat16


def build_nc(causal: bool, reps: int = 0) -> bacc.Bacc:
    """reps>0 wraps the whole body in a hardware loop (for on-device timing)."""
    nc = bacc.Bacc("TRN2", target_bir_lowering=False, debug=False, num_devices=NCORE)

    xT = nc.declare_dram_parameter("xT", [D, S], F32R, isOutput=False)
    kxT = nc.declare_dram_parameter("kxT", [D, S], F32R, isOutput=False)
    vxT = nc.declare_dram_parameter("vxT", [D, S], F32R, isOutput=False)
    wv = nc.declare_dram_parameter("wv", [D, HPC * DH], F32R, isOutput=False)
    wqk = nc.declare_dram_parameter("wqk", [2, D, HPC * DH], F32R, isOutput=False)
    # constants blob: [128, 136] = tri(0:128) | bqT(128:130) | bkT(130:132) | bvT(132:136)
    cst = nc.declare_dram_parameter("cst", [128, 136], F32, isOutput=False)
    out = nc.declare_dram_parameter("out", [HPC, DH, S], F32, isOutput=True)

    NDC = D // 128  # 4 d-chunks

    from contextlib import ExitStack
    with tile.TileContext(nc) as tc, ExitStack() as _st:
        persist = _st.enter_context(tc.tile_pool(name="persist", bufs=1))
        wpool = _st.enter_context(tc.tile_pool(name="wtile", bufs=8))
        rpool = _st.enter_context(tc.tile_pool(name="res", bufs=3))
        eppool = _st.enter_context(tc.tile_pool(name="eptmp", bufs=2))
        ps_sc = _st.enter_context(tc.tile_pool(name="ps_sc", bufs=3, space="PSUM"))
        ps_out = _st.enter_context(tc.tile_pool(name="ps_out", bufs=2, space="PSUM"))
        if reps:
            _st.enter_context(tc.For_i(0, reps, 1))
        if True:
            # ---- constants + weights: consolidated single DMAs ----
            cst_sb = persist.tile([128, 136], F32, tag="cst")
            nc.scalar.dma_start(out=cst_sb[:], in_=cst[:])
            tri_sb = cst_sb[:, 0:KT]
            bq_sb = cst_sb[:, 128:130]
            bk_sb = cst_sb[:, 130:132]
            bv_sb = cst_sb[0:DH, 132:136]

            wv_sb = persist.tile([128, NDC, HPC * DH], F32R, tag="wv")
            nc.scalar.dma_start(
                out=wv_sb[:], in_=wv.rearrange("(dc p) j -> p dc j", p=128))
            # wqk gates every projection matmul: first on the sync queue
            wqk_sb = persist.tile([128, 2, NDC, HPC * DH], F32R, tag="wqk")
            nc.sync.dma_start(
                out=wqk_sb[:], in_=wqk.rearrange("t (dc p) j -> p t dc j", p=128))
            wq_sb = wqk_sb[:, 0]
            wk_sb = wqk_sb[:, 1]

            x_sb = persist.tile([128, NDC, S], F32R, tag="x")
            kx_sb = persist.tile([128, NDC, S], F32R, tag="kx")
            qT_sb = persist.tile([128, 2, S], BF16, tag="qT")
            kT_sb = object()  # sentinel for the eviction branch
            # per-head K-padded key tiles: complement rows are zero so
            # scores matmuls run at K=128 (fast weight-load path)
            kTz_sb = persist.tile([128, HPC, S], BF16, tag="kTz")
            v_sb = persist.tile([128, S // 128, HPC, DH + 1], BF16, tag="v")


            vxpool = _st.enter_context(tc.tile_pool(name="vxp", bufs=2))
            _vq = {}

            def vx_dma(qi):
                # DMA one 512-seq quarter of vx (issued early; projected later)
                vq = vxpool.tile([128, NDC, 512], F32R, tag="vxs")
                _vq[qi] = vq
                vxr = vxT.rearrange("(dc p) s -> p dc s", p=128)
                nc.sync.dma_start(out=vq[:], in_=vxr[:, :, qi * 512:(qi + 1) * 512])

            def v_proj(qi):
                vq = _vq.pop(qi)
                for st4 in range(4):
                    st = qi * 4 + st4
                    ps = ps_sc.tile([128, SW], F32, tag="sc")
                    for dc in range(NDC):
                        nc.tensor.matmul(
                            ps[:, 0:HPC * DH],
                            vq[:, dc, st4 * 128:(st4 + 1) * 128],
                            wv_sb[:, dc, :],
                            start=(dc == 0),
                            stop=(dc == NDC - 1),
                        )
                    nc.vector.tensor_copy(
                        out=v_sb[:, st, :, 0:DH],
                        in_=ps[:, 0:HPC * DH].rearrange("p (u d) -> p u d", u=HPC),
                    )

            def proj_dma(s0):
                xr = xT.rearrange("(dc p) s -> p dc s", p=128)
                kxr = kxT.rearrange("(dc p) s -> p dc s", p=128)
                for q in range(s0, s0 + SW, 512):
                    nc.sync.dma_start(out=x_sb[:, :, q:q + 512], in_=xr[:, :, q:q + 512])
                    nc.sync.dma_start(out=kx_sb[:, :, q:q + 512], in_=kxr[:, :, q:q + 512])

            def proj_half(s0):
                # g outer: head-group 0's q AND k finish first (they gate
                # the first two attention units)
                for g in range(2):
                    for (w_sb, b_sb, src, dst) in (
                        (wq_sb, bq_sb, x_sb, qT_sb), (wk_sb, bk_sb, kx_sb, kT_sb)
                    ):
                        ps = ps_sc.tile([128, SW], F32, tag="sc")
                        for nchunk in range(0, SW, 512):
                            for dc in range(NDC):
                                nc.tensor.matmul(
                                    ps[:, nchunk:nchunk + 512],
                                    w_sb[:, dc, g * 128:(g + 1) * 128],
                                    src[:, dc, s0 + nchunk:s0 + nchunk + 512],
                                    start=(dc == 0),
                                    stop=(dc == NDC - 1),
                                )
                            # per-512 bias add releases qT/kT chunks early
                            ch = slice(s0 + nchunk, s0 + nchunk + 512)
                            if dst is kT_sb:
                                # k rows land in the SAME row range as the
                                # head's q rows; complement rows are zero
                                for ho in range(2):
                                    rs = slice(ho * DH, (ho + 1) * DH)
                                    nc.vector.tensor_scalar_add(
                                        out=kTz_sb[rs, 2 * g + ho, ch],
                                        in0=ps[rs, nchunk:nchunk + 512],
                                        scalar1=b_sb[rs, g:g + 1],
                                    )
                            else:
                                nc.vector.tensor_scalar_add(
                                    out=dst[:, g, ch],
                                    in0=ps[:, nchunk:nchunk + 512],
                                    scalar1=b_sb[:, g:g + 1],
                                )

            def attn_sweep(u, sw):
                g, ho = u // 2, u % 2
                qh = qT_sb[:, g, :]       # [128, S]; rows 64+ hit zero weights
                kh = kTz_sb[:, u, :]      # [128, S] zero-padded per head
                q0 = sw * SW
                qw = SW
                nkt = (q0 + qw) // KT if causal else S // KT
                ncc = qw // 512
                # one accumulator (1 PSUM bank) per 512-q-chunk: earlier chunks
                # finish at earlier k-tiles, freeing slots sooner
                o_ps = []
                for _cc in range(ncc):
                    o_chunk = ps_out.tile([DH + 1, 512], F32, tag="out")
                    o_ps.append(o_chunk)
                # last k-tile contributing to each 512-chunk of the sweep
                last_kt = [
                    min(nkt - 1, (q0 + ch + 512 - 1) // KT)
                    for ch in range(0, qw, 512)
                ] if causal else [nkt - 1] * (qw // 512)
                def emit_av(kt, w, a0):
                    for c0 in range(a0, qw, 512):
                        c1 = min(c0 + 512, qw)
                        nc.tensor.matmul(
                            o_ps[c0 // 512][:, 0:c1 - c0],
                            v_sb[:, kt, u, :],
                            w[:, c0:c1],
                            start=(kt == 0),
                            stop=(kt == last_kt[c0 // 512]),
                        )

                pend = None   # software-pipeline AV one k-tile behind scores
                for kt in range(nkt):
                    o = max(0, kt * KT - q0) if causal else 0
                    a0 = (o // 512) * 512              # 512-aligned start for AV
                    sc = ps_sc.tile([128, SW], F32, tag="sc")
                    c0 = o
                    while c0 < qw:
                        c1 = min(((c0 // 512) + 1) * 512, qw)
                        nc.tensor.matmul(
                            sc[:, c0:c1],
                            kh[:, kt * KT:(kt + 1) * KT],
                            qh[:, q0 + c0:q0 + c1],
                            start=True, stop=True,
                        )
                        c0 = c1
                    if causal and kt * KT >= q0:
                        # diagonal block: cols [o, o+128)
                        nc.vector.tensor_add(
                            out=sc[:, o:o + KT], in0=sc[:, o:o + KT], in1=tri_sb[:]
                        )
                    w = wpool.tile([128, SW], BF16, tag="w")
                    if o > a0:
                        nc.gpsimd.memset(w[:, a0:o], 0.0)
                    nc.scalar.activation(
                        out=w[:, o:qw], in_=sc[:, o:qw],
                        func=mybir.ActivationFunctionType.Exp, scale=0.125,
                    )
                    if pend is not None:
                        emit_av(*pend)
                    pend = (kt, w, a0)
                if pend is not None:
                    emit_av(*pend)
                # epilogue per chunk: divide by denoms (row 64) + bias, DMA out
                for cc in range(ncc):
                    op = o_ps[cc]
                    r65 = eppool.tile([DH + 1, 512], F32, tag="r65")
                    nc.vector.reciprocal(out=r65[DH:DH + 1, :], in_=op[DH:DH + 1, :])
                    r0 = eppool.tile([1, 512], F32, tag="r0")
                    nc.sync.dma_start(out=r0[:], in_=r65[DH:DH + 1, :])
                    db = eppool.tile([DH, 512], F32, tag="db")
                    nc.gpsimd.partition_broadcast(db[:], r0[:])
                    res = rpool.tile([DH, 512], F32, tag="res")
                    nc.vector.tensor_mul(out=res[:], in0=op[0:DH, :], in1=db[:])
                    nc.vector.tensor_scalar_add(
                        out=res[:], in0=res[:], scalar1=bv_sb[:, u:u + 1])
                    nc.sync.dma_start(
                        out=out[u, :, q0 + cc * 512:q0 + (cc + 1) * 512], in_=res[:])

            # sweep 0 only needs the first half of qT/kT: interleave so
            # attention starts while half-1 inputs are still in flight.
            # v ones column (bv added at the very end)
            nc.vector.memset(v_sb[:, :, :, DH], 1.0)
            for _u in range(HPC):
                _zr = slice(DH, 128) if _u % 2 == 0 else slice(0, DH)
                nc.gpsimd.memset(kTz_sb[_zr, _u, :], 0.0)
            if causal:
                # sweep 0 needs only half-0 of q/k/v: start attention while
                # half-1 inputs are still in flight
                proj_dma(0)
                proj_half(0)
                vx_dma(0)
                v_proj(0)
                vx_dma(1)
                v_proj(1)        # v for k-tiles 0..7 (all sweep-0 needs)
                proj_dma(SW)     # enqueue ALL remaining input loads before
                vx_dma(2)        # any compute-gated epilogue DMA
                vx_dma(3)
                attn_sweep(0, 0)
                attn_sweep(1, 0)
                proj_half(SW)
                attn_sweep(2, 0)
                attn_sweep(3, 0)
                v_proj(2)
                v_proj(3)        # v for k-tiles 8..15 (sweep 1)
                for u in range(HPC):
                    attn_sweep(u, 1)
            else:
                # full attention: every sweep needs all of k/v first
                proj_dma(0)
                proj_half(0)
                proj_dma(SW)
                for qi in range(4):
                    vx_dma(qi)
                    v_proj(qi)
                proj_half(SW)
                for sw in range(NSW):
                    for u in range(HPC):
                        attn_sweep(u, sw)

    nc.finalize()
    return nc


_NC_CACHE = {}


def _get_nc(causal: bool):
    if causal not in _NC_CACHE:
        _NC_CACHE[causal] = build_nc(causal)
    return _NC_CACHE[causal]


def make_in_maps(input_tensor, keys_vector, values_vector, Wq, bq, Wk, bk, Wv, bv):
    # scores tiles are [k, q] (transposed): keep k <= q  ->  upper triangle
    tri_np = np.where(
        np.triu(np.ones((KT, KT), dtype=bool)), np.float32(0), NEG
    ).astype(np.float32)
    in_maps = []
    for c in range(NCORE):
        b, hg = c // 2, c % 2
        hs = slice(hg * HPC * DH, (hg + 1) * HPC * DH)
        cst = np.zeros((128, 136), np.float32)
        cst[:, 0:128] = tri_np
        cst[:, 128:130] = np.asarray(bq)[hs].reshape(2, 128).T
        cst[:, 130:132] = np.asarray(bk)[hs].reshape(2, 128).T
        cst[0:DH, 132:136] = np.asarray(bv)[hs].reshape(HPC, DH).T
        m = {
            "xT": np.ascontiguousarray(np.asarray(input_tensor)[:, b, :].T),
            "kxT": np.ascontiguousarray(np.asarray(keys_vector)[:, b, :].T),
            "vxT": np.ascontiguousarray(np.asarray(values_vector)[:, b, :].T),
            "wv": np.ascontiguousarray(np.asarray(Wv)[:, hs]),
            "wqk": np.ascontiguousarray(
                np.stack([np.asarray(Wq)[:, hs], np.asarray(Wk)[:, hs]])),
            "cst": cst,
        }
        in_maps.append(m)
    return in_maps


def assemble_output(results):
    full = np.empty((S, B, D), dtype=np.float32)
    for c in range(NCORE):
        b, hg = c // 2, c % 2
        o = results[c]["out"]  # [HPC, DH, S]
        for u in range(HPC):
            h = hg * HPC + u
            full[:, b, h * DH:(h + 1) * DH] = o[u].T
    return full


def kernel(input_tensor, keys_vector, values_vector, Wq, bq, Wk, bk, Wv, bv, mask):
    causal = bool(np.asarray(mask).item()) if np.asarray(mask).size == 1 else True
    nc = _get_nc(causal)
    in_maps = make_in_maps(
        input_tensor, keys_vector, values_vector, Wq, bq, Wk, bk, Wv, bv
    )
    res = run_bass_kernel_spmd(nc, in_maps, core_ids=list(range(NCORE)))
    return assemble_output(res.results)


# revision 60
# speedup vs baseline: 1.1590x; 1.0476x over previous
"""Distributed causal multi-head attention layer for one TRN2 chip (8 NeuronCores).

Problem: S=2048, B=4, D=512, H=8 heads (DH=64), causal mask, fp32 I/O.

Sharding: core c handles batch b = c//2 and heads [4*(c%2), 4*(c%2)+4).
Each core computes its 4 heads' attention for its batch; the host
concatenates per-core outputs (no cross-core collectives needed).

Per-core kernel (Tile framework), flash-attention style without max-subtraction
(scores ~ N(0,1), fp32 exp cannot overflow):
  - QKV projections on TensorE in float32r (full-rate fp32): qT in [dh, seq]
    layout (2 heads per 128 partitions), per-head zero-padded kTz (bf16), and
    v in [seq, dh] bf16 with a ones-column at col 64.
  - K is zero-padded to 128 per head (complement rows zero) so scores matmuls
    run at K=128: the K=64 fp32r path measured 507 ns/matmul on HW vs 365 for
    K=128 bf16 (LDWEIGHTS fast-path).
  - Attention per head, q swept in 2 half-rows of 1024, k-tiles of 128:
      scoresT[k,q] = kTz_head x qT (PE, bf16, fp32 PSUM)
      causal tri-mask add on the diagonal 128x128 block (DVE)
      w = exp(scores/8) (ScalarE, PSUM -> bf16 SBUF)
      out_aug[65, 512-chunk] += v_aug.T @ w (PE; row 64 = softmax denominator)
  - Epilogue per 512-chunk: reciprocal (DVE) -> DMA shift to partition 0 ->
    partition_broadcast (GPSIMD reads physical partition 0 only!) -> multiply
    + bias add (DVE) -> DMA out in [dh, seq] layout.
  - DMA choreography: weights first, then x/kx half-0 quarters, vx, half-1 --
    all input DMAs enqueue on the sync queue before any compute-gated epilogue
    DMA (FIFO inversion otherwise delays half-1 inputs by ~10 us).
Host transposes/concats per-head blocks into the full [S, B, D] output.
reps>0 wraps the body in a hardware For_i loop for on-device timing.
"""

import numpy as np

import concourse.bass as bass
import concourse.tile as tile
from concourse import bacc, mybir
from concourse.bass_utils import run_bass_kernel_spmd

S, B, D, H = 2048, 4, 512, 8
DH = D // H            # 64
HPC = 4                # heads per core
NCORE = 8
SW = 1024              # q sweep width
NSW = S // SW          # 2
KT = 128               # key tile (partition dim)
NEG = np.float32(-1e9)

F32 = mybir.dt.float32
F32R = mybir.dt.float32r
BF16 = mybir.dt.bfloat16


def build_nc(causal: bool, reps: int = 0) -> bacc.Bacc:
    """reps>0 wraps the whole body in a hardware loop (for on-device timing)."""
    nc = bacc.Bacc("TRN2", target_bir_lowering=False, debug=False, num_devices=NCORE)

    xT = nc.declare_dram_parameter("xT", [D, S], F32R, isOutput=False)
    kxT = nc.declare_dram_parameter("kxT", [D, S], F32R, isOutput=False)
    vxT = nc.declare_dram_parameter("vxT", [D, S], F32R, isOutput=False)
    wv = nc.declare_dram_parameter("wv", [D, HPC * DH], F32R, isOutput=False)
    wqk = nc.declare_dram_parameter("wqk", [2, D, HPC * DH], F32R, isOutput=False)
    # constants blob: [128, 136] = tri(0:128) | bqT(128:130) | bkT(130:132) | bvT(132:136)
    cst = nc.declare_dram_parameter("cst", [128, 136], F32, isOutput=False)
    out = nc.declare_dram_parameter("out", [HPC, DH, S], F32, isOutput=True)

    NDC = D // 128  # 4 d-chunks

    from contextlib import ExitStack
    with tile.TileContext(nc) as tc, ExitStack() as _st:
        persist = _st.enter_context(tc.tile_pool(name="persist", bufs=1))
        wpool = _st.enter_context(tc.tile_pool(name="wtile", bufs=8))
        rpool = _st.enter_context(tc.tile_pool(name="res", bufs=3))
        eppool = _st.enter_context(tc.tile_pool(name="eptmp", bufs=2))
        ps_sc = _st.enter_context(tc.tile_pool(name="ps_sc", bufs=3, space="PSUM"))
        ps_out = _st.enter_context(tc.tile_pool(name="ps_out", bufs=2, space="PSUM"))
        if reps:
            _st.enter_context(tc.For_i(0, reps, 1))
        if True:
            # ---- constants + weights: consolidated single DMAs ----
            cst_sb = persist.tile([128, 136], F32, tag="cst")
            nc.scalar.dma_start(out=cst_sb[:], in_=cst[:])
            tri_sb = cst_sb[:, 0:KT]
            bq_sb = cst_sb[:, 128:130]
            bk_sb = cst_sb[:, 130:132]
            bv_sb = cst_sb[0:DH, 132:136]

            wv_sb = persist.tile([128, NDC, HPC * DH], F32R, tag="wv")
            nc.scalar.dma_start(
                out=wv_sb[:], in_=wv.rearrange("(dc p) j -> p dc j", p=128))
            # wqk gates every projection matmul: first on the sync queue,
            # split so head-group 0 (first attention units) arrives earliest
            wqk_sb = persist.tile([128, 2, NDC, HPC * DH], F32R, tag="wqk")
            wqkr = wqk.rearrange("t (dc p) j -> p t dc j", p=128)
            for g in range(2):
                for t in range(2):
                    nc.sync.dma_start(
                        out=wqk_sb[:, t, :, g * 128:(g + 1) * 128],
                        in_=wqkr[:, t, :, g * 128:(g + 1) * 128])
            wq_sb = wqk_sb[:, 0]
            wk_sb = wqk_sb[:, 1]

            x_sb = persist.tile([128, NDC, S], F32R, tag="x")
            kx_sb = persist.tile([128, NDC, S], F32R, tag="kx")
            qT_sb = persist.tile([128, 2, S], BF16, tag="qT")
            kT_sb = object()  # sentinel for the eviction branch
            # per-head K-padded key tiles: complement rows are zero so
            # scores matmuls run at K=128 (fast weight-load path)
            kTz_sb = persist.tile([128, HPC, S], BF16, tag="kTz")
            v_sb = persist.tile([128, S // 128, HPC, DH + 1], BF16, tag="v")


            vxpool = _st.enter_context(tc.tile_pool(name="vxp", bufs=2))
            _vq = {}

            def vx_dma(qi):
                # DMA one 512-seq quarter of vx (issued early; projected later)
                vq = vxpool.tile([128, NDC, 512], F32R, tag="vxs")
                _vq[qi] = vq
                vxr = vxT.rearrange("(dc p) s -> p dc s", p=128)
                nc.sync.dma_start(out=vq[:], in_=vxr[:, :, qi * 512:(qi + 1) * 512])

            def v_proj(qi):
                vq = _vq.pop(qi)
                for st4 in range(4):
                    st = qi * 4 + st4
                    ps = ps_sc.tile([128, SW], F32, tag="sc")
                    for dc in range(NDC):
                        nc.tensor.matmul(
                            ps[:, 0:HPC * DH],
                            vq[:, dc, st4 * 128:(st4 + 1) * 128],
                            wv_sb[:, dc, :],
                            start=(dc == 0),
                            stop=(dc == NDC - 1),
                        )
                    nc.vector.tensor_copy(
                        out=v_sb[:, st, :, 0:DH],
                        in_=ps[:, 0:HPC * DH].rearrange("p (u d) -> p u d", u=HPC),
                    )

            def proj_dma(s0):
                xr = xT.rearrange("(dc p) s -> p dc s", p=128)
                kxr = kxT.rearrange("(dc p) s -> p dc s", p=128)
                for q in range(s0, s0 + SW, 512):
                    nc.sync.dma_start(out=x_sb[:, :, q:q + 512], in_=xr[:, :, q:q + 512])
                    nc.sync.dma_start(out=kx_sb[:, :, q:q + 512], in_=kxr[:, :, q:q + 512])

            def proj_half(s0):
                # g outer: head-group 0's q AND k finish first (they gate
                # the first two attention units)
                for g in range(2):
                    for (w_sb, b_sb, src, dst) in (
                        (wq_sb, bq_sb, x_sb, qT_sb), (wk_sb, bk_sb, kx_sb, kT_sb)
                    ):
                        ps = ps_sc.tile([128, SW], F32, tag="sc")
                        for nchunk in range(0, SW, 512):
                            for dc in range(NDC):
                                nc.tensor.matmul(
                                    ps[:, nchunk:nchunk + 512],
                                    w_sb[:, dc, g * 128:(g + 1) * 128],
                                    src[:, dc, s0 + nchunk:s0 + nchunk + 512],
                                    start=(dc == 0),
                                    stop=(dc == NDC - 1),
                                )
                            # per-512 bias add releases qT/kT chunks early
                            ch = slice(s0 + nchunk, s0 + nchunk + 512)
                            if dst is kT_sb:
                                # k rows land in the SAME row range as the
                                # head's q rows; complement rows are zero
                                for ho in range(2):
                                    rs = slice(ho * DH, (ho + 1) * DH)
                                    nc.vector.tensor_scalar_add(
                                        out=kTz_sb[rs, 2 * g + ho, ch],
                                        in0=ps[rs, nchunk:nchunk + 512],
                                        scalar1=b_sb[rs, g:g + 1],
                                    )
                            else:
                                nc.vector.tensor_scalar_add(
                                    out=dst[:, g, ch],
                                    in0=ps[:, nchunk:nchunk + 512],
                                    scalar1=b_sb[:, g:g + 1],
                                )

            def attn_sweep(u, sw):
                g, ho = u // 2, u % 2
                qh = qT_sb[:, g, :]       # [128, S]; rows 64+ hit zero weights
                kh = kTz_sb[:, u, :]      # [128, S] zero-padded per head
                q0 = sw * SW
                qw = SW
                nkt = (q0 + qw) // KT if causal else S // KT
                ncc = qw // 512
                # one accumulator (1 PSUM bank) per 512-q-chunk: earlier chunks
                # finish at earlier k-tiles, freeing slots sooner
                o_ps = []
                for _cc in range(ncc):
                    o_chunk = ps_out.tile([DH + 1, 512], F32, tag="out")
                    o_ps.append(o_chunk)
                # last k-tile contributing to each 512-chunk of the sweep
                last_kt = [
                    min(nkt - 1, (q0 + ch + 512 - 1) // KT)
                    for ch in range(0, qw, 512)
                ] if causal else [nkt - 1] * (qw // 512)
                def emit_av(kt, w, a0):
                    for c0 in range(a0, qw, 512):
                        c1 = min(c0 + 512, qw)
                        nc.tensor.matmul(
                            o_ps[c0 // 512][:, 0:c1 - c0],
                            v_sb[:, kt, u, :],
                            w[:, c0:c1],
                            start=(kt == 0),
                            stop=(kt == last_kt[c0 // 512]),
                        )

                pend = None   # software-pipeline AV one k-tile behind scores
                for kt in range(nkt):
                    o = max(0, kt * KT - q0) if causal else 0
                    a0 = (o // 512) * 512              # 512-aligned start for AV
                    sc = ps_sc.tile([128, SW], F32, tag="sc")
                    c0 = o
                    while c0 < qw:
                        c1 = min(((c0 // 512) + 1) * 512, qw)
                        nc.tensor.matmul(
                            sc[:, c0:c1],
                            kh[:, kt * KT:(kt + 1) * KT],
                            qh[:, q0 + c0:q0 + c1],
                            start=True, stop=True,
                        )
                        c0 = c1
                    if causal and kt * KT >= q0:
                        # diagonal block: cols [o, o+128)
                        nc.vector.tensor_add(
                            out=sc[:, o:o + KT], in0=sc[:, o:o + KT], in1=tri_sb[:]
                        )
                    w = wpool.tile([128, SW], BF16, tag="w")
                    if o > a0:
                        nc.gpsimd.memset(w[:, a0:o], 0.0)
                    nc.scalar.activation(
                        out=w[:, o:qw], in_=sc[:, o:qw],
                        func=mybir.ActivationFunctionType.Exp, scale=0.125,
                    )
                    if pend is not None:
                        emit_av(*pend)
                    pend = (kt, w, a0)
                if pend is not None:
                    emit_av(*pend)
                # epilogue per chunk: divide by denoms (row 64) + bias, DMA out.
                # Numerator is evicted to SBUF right away so the PSUM slot is
                # freed before the slow shift/broadcast chain (the next unit's
                # AV matmuls wait on these slots).
                for cc in range(ncc):
                    op = o_ps[cc]
                    r65 = eppool.tile([DH + 1, 512], F32, tag="r65")
                    nc.vector.reciprocal(out=r65[DH:DH + 1, :], in_=op[DH:DH + 1, :])
                    res = rpool.tile([DH, 512], F32, tag="res")
                    nc.vector.tensor_copy(out=res[:], in_=op[0:DH, :])
                    r0 = eppool.tile([1, 512], F32, tag="r0")
                    nc.sync.dma_start(out=r0[:], in_=r65[DH:DH + 1, :])
                    db = eppool.tile([DH, 512], F32, tag="db")
                    nc.gpsimd.partition_broadcast(db[:], r0[:])
                    nc.vector.tensor_mul(out=res[:], in0=res[:], in1=db[:])
                    nc.vector.tensor_scalar_add(
                        out=res[:], in0=res[:], scalar1=bv_sb[:, u:u + 1])
                    nc.sync.dma_start(
                        out=out[u, :, q0 + cc * 512:q0 + (cc + 1) * 512], in_=res[:])

            # sweep 0 only needs the first half of qT/kT: interleave so
            # attention starts while half-1 inputs are still in flight.
            # v ones column (bv added at the very end)
            nc.vector.memset(v_sb[:, :, :, DH], 1.0)
            for _u in range(HPC):
                _zr = slice(DH, 128) if _u % 2 == 0 else slice(0, DH)
                nc.gpsimd.memset(kTz_sb[_zr, _u, :], 0.0)
            if causal:
                # sweep 0 needs only half-0 of q/k/v: start attention while
                # half-1 inputs are still in flight
                proj_dma(0)
                proj_half(0)
                vx_dma(0)
                v_proj(0)
                vx_dma(1)
                v_proj(1)        # v for k-tiles 0..7 (all sweep-0 needs)
                proj_dma(SW)     # enqueue ALL remaining input loads before
                vx_dma(2)        # any compute-gated epilogue DMA
                vx_dma(3)
                attn_sweep(0, 0)
                attn_sweep(1, 0)
                proj_half(SW)
                attn_sweep(2, 0)
                attn_sweep(3, 0)
                v_proj(2)
                v_proj(3)        # v for k-tiles 8..15 (sweep 1)
                for u in range(HPC):
                    attn_sweep(u, 1)
            else:
                # full attention: every sweep needs all of k/v first
                proj_dma(0)
                proj_half(0)
                proj_dma(SW)
                for qi in range(4):
                    vx_dma(qi)
                    v_proj(qi)
                proj_half(SW)
                for sw in range(NSW):
                    for u in range(HPC):
                        attn_sweep(u, sw)

    nc.finalize()
    return nc


_NC_CACHE = {}


def _get_nc(causal: bool):
    if causal not in _NC_CACHE:
        _NC_CACHE[causal] = build_nc(causal)
    return _NC_CACHE[causal]


def make_in_maps(input_tensor, keys_vector, values_vector, Wq, bq, Wk, bk, Wv, bv):
    # scores tiles are [k, q] (transposed): keep k <= q  ->  upper triangle
    tri_np = np.where(
        np.triu(np.ones((KT, KT), dtype=bool)), np.float32(0), NEG
    ).astype(np.float32)
    in_maps = []
    for c in range(NCORE):
        b, hg = c // 2, c % 2
        hs = slice(hg * HPC * DH, (hg + 1) * HPC * DH)
        cst = np.zeros((128, 136), np.float32)
        cst[:, 0:128] = tri_np
        cst[:, 128:130] = np.asarray(bq)[hs].reshape(2, 128).T
        cst[:, 130:132] = np.asarray(bk)[hs].reshape(2, 128).T
        cst[0:DH, 132:136] = np.asarray(bv)[hs].reshape(HPC, DH).T
        m = {
            "xT": np.ascontiguousarray(np.asarray(input_tensor)[:, b, :].T),
            "kxT": np.ascontiguousarray(np.asarray(keys_vector)[:, b, :].T),
            "vxT": np.ascontiguousarray(np.asarray(values_vector)[:, b, :].T),
            "wv": np.ascontiguousarray(np.asarray(Wv)[:, hs]),
            "wqk": np.ascontiguousarray(
                np.stack([np.asarray(Wq)[:, hs], np.asarray(Wk)[:, hs]])),
            "cst": cst,
        }
        in_maps.append(m)
    return in_maps


def assemble_output(results):
    full = np.empty((S, B, D), dtype=np.float32)
    for c in range(NCORE):
        b, hg = c // 2, c % 2
        o = results[c]["out"]  # [HPC, DH, S]
        for u in range(HPC):
            h = hg * HPC + u
            full[:, b, h * DH:(h + 1) * DH] = o[u].T
    return full


def kernel(input_tensor, keys_vector, values_vector, Wq, bq, Wk, bk, Wv, bv, mask):
    causal = bool(np.asarray(mask).item()) if np.asarray(mask).size == 1 else True
    nc = _get_nc(causal)
    in_maps = make_in_maps(
        input_tensor, keys_vector, values_vector, Wq, bq, Wk, bk, Wv, bv
    )
    res = run_bass_kernel_spmd(nc, in_maps, core_ids=list(range(NCORE)))
    return assemble_output(res.results)
